# revision 4
# baseline (speedup 1.0000x reference)
"""Trainium2 Bass kernel for nn_ComputeEdgeLoss.

Computes, for each batch b and lower-triangular pair (i, j) of the 64
recon keypoints, the mean over 5 interpolated segment points of the min
squared distance to the 2048 gt points of that batch.

Strategy (v3)
-------------
Sharding: 8 cores = 4 batches x 2 pair-halves (1008 pairs each); the 64
endpoint queries of a batch are split 32/32 between its two cores, so
each core owns exactly 3056 query rows -> 24 row-tiles of 128.  gt
replicated per batch (sharding_hint).

Math: for a query point k and gt point g,
    ||k - g||^2 = a . b,  a = [kx, ky, kz, ||k||^2, 1],
                          b = [-2gx, -2gy, -2gz, 1, ||g||^2]
so PE matmuls produce [128 x 2048] tiles of squared distances in PSUM.
Precision: fp32 inputs are split into three bf16 terms x ~= h + l + r
and the six product groups >= 2^-24 (hh, hl, lh, hr, rh, ll) become
K=30 bf16 contraction rows (padded to 32).  ~3e-4 max rel err measured.

PE array packing: K=32 <= 32 enables 4x row tiling -- four 32x128 bands
(tile_position=(32q, 0)) hold different stationary pair-blocks, each
streaming its own gt chunk.  Operands for band q live on SBUF
partitions [32q, 32q+32).

Drain (the bottleneck; walrus forbids dual-PSUM-operand TT, raw-ISA DVE
ops, Pool-engine tensor ops, and DMA min-accum, so only ScalarE+DVE can
touch the 6.3M PSUM values per core):
 - A-tiles (1 in 4): DVE fp32 tensor_reduce(min) straight from PSUM,
   [128,2048] in one flat instruction (~2290ns measured).
 - B-tiles (3 in 4): ScalarE casts [128,2048] to fp16 in SBUF
   (~1967ns), then ONE flat fp16 tensor_reduce at 2x DVE rate
   (~1223ns).  Flat APs only: 3D k-batched APs measured at half rate,
   so no fold pyramids.
Groups of (A,B,B,B) self-balance ScalarE (3x1967=5.9us) against DVE
(2290+3x1223=5.96us).  PE (~25us) hides underneath; PSUM pool of 4
big tiles keeps it streaming.  A dummy 1-col ScalarE copy at t=0 pulls
the 1.3us ACT table load off the critical path, and the input DMA is
ordered so tile 0's operands land first.
"""

import numpy as np

import concourse.bass as bass
import concourse.mybir as mybir
import concourse.tile as tile
from concourse.bass_utils import run_bass_kernel_spmd

# Problem shape (hardcoded per contest rules).
B = 4          # batches
NPTS = 64      # recon points per batch
M = 2048       # gt points per batch
P = NPTS * (NPTS - 1) // 2   # 2016 pairs
HALF = P // 2                # 1008 pairs per core
N_CORES = 8
FRACS = (0.25, 0.5, 0.75)    # interior interpolation fractions
NF = len(FRACS)
NINT = NF * HALF             # 3024 interior rows per core
NEND = 32                    # endpoint rows per core (64 split 32/32)
NTILES = 24                  # row-tiles of 128 (3072 rows, 3056 used)
ROWS = NTILES * 128
KEXT = 32                    # padded contraction depth (30 real rows)
PFCOLS = (NTILES // 2) * 128  # 1536: even tiles on partitions 0-63, odd on 64-127
ABCOLS = M + PFCOLS

_II, _JJ = np.tril_indices(NPTS, -1)   # pair order matches reference

# Tiles drained by DVE directly from PSUM (A); the rest are ScalarE-
# staged to fp16 then DVE-reduced (B).  Pattern A,B,B,B per group of 4:
# DVE gets PSUM work the moment tile 0 lands while ScalarE copies the
# following tiles; each group is engine-balanced.
A_SET = (0, 4, 8, 12, 16, 20)


def _split3_bf16(x: np.ndarray):
    """Split fp32 x into three bf16 terms with x ~= h + l + r (27-bit
    significand fidelity; differences are Sterbenz-exact in fp32)."""
    import ml_dtypes

    bf16 = ml_dtypes.bfloat16
    x = np.ascontiguousarray(x, dtype=np.float32)
    h = x.astype(bf16)
    l32 = (x - h.astype(np.float32)).astype(np.float32)
    l = l32.astype(bf16)
    r = (l32 - l.astype(np.float32)).astype(np.float32).astype(bf16)
    return h, l, r


_COMPUTE_ENGINES = {"PE", "DVE", "Activation", "Pool"}


def _prune_redundant_waits(bir: dict) -> dict:
    """Reduce every instruction to at most ONE sync-wait.

    This walrus build accepts only one sync-wait per instruction, but
    Tile's semaphore pass is not transitively minimal.  We reconstruct
    per-instruction guaranteed semaphore lower bounds (vector clocks
    over the scheduled program order) and delete implied waits; any
    residual multi-wait instruction is split into single-wait Drain
    carriers on the same engine.

    Soundness model: per-engine in-order dispatch; in-order completion
    for compute engines; per-semaphore in-order completion for DMA-queue
    sems (each DMAHW sem belongs to one queue).  Only monotone
    (inc-only) semaphores with sem-ge-imm waits participate.
    """
    fn = bir["functions"][0]

    # Semaphore properties across the whole program.
    contrib_engines: dict[int, set] = {}
    monotone: dict[int, bool] = {}
    for b in fn["blocks"]:
        for ins in b["instructions"]:
            sy = ins.get("sync_info") or {}
            for u in sy.get("on_update") or []:
                if u.get("sync_type") != "semaphore":
                    continue
                s = u["id"]
                contrib_engines.setdefault(s, set()).add(ins.get("engine"))
                ok = u.get("update_mode") == "sem-inc"
                monotone[s] = monotone.get(s, True) and ok

    def usable(s):
        return monotone.get(s, False)

    def mergemax(dst, src):
        for k, v in src.items():
            if dst.get(k, -1) < v:
                dst[k] = v

    prev_start_know: dict[str, dict] = {}
    cum: dict[int, int] = {}            # sem -> cumulative inc in walk order
    comp_know: list[dict] = []          # per walk index
    sem_reach: dict[int, list] = {}     # sem -> [(value_after, walk_idx)]
    dropped = 0
    walk_idx = 0

    for b in fn["blocks"]:
        new_insts = []
        for ins in b["instructions"]:
            eng = ins.get("engine")
            sy = ins.get("sync_info") or {}
            waits = list(sy.get("on_wait") or [])

            def know_from(wlist):
                know = dict(prev_start_know.get(eng, {}))
                for w in wlist:
                    if (w.get("sync_type") != "semaphore"
                            or w.get("wait_mode") != "sem-ge-imm"):
                        continue
                    s, v = w["id"], w["wait_value"]
                    if not usable(s):
                        continue
                    if know.get(s, -1) < v:
                        know[s] = v
                    if len(contrib_engines.get(s, ())) == 1:
                        for after, pidx in sem_reach.get(s, ()):
                            if after >= v:
                                mergemax(know, comp_know[pidx])
                                break
                return know

            if len(waits) > 1:
                kept = list(waits)
                changed = True
                while changed and len(kept) > 1:
                    changed = False
                    for w in list(kept):
                        others = [x for x in kept if x is not w]
                        if (w.get("sync_type") == "semaphore"
                                and w.get("wait_mode") == "sem-ge-imm"
                                and usable(w["id"])
                                and know_from(others).get(w["id"], -1)
                                >= w["wait_value"]):
                            kept.remove(w)
                            dropped += 1
                            changed = True
                            break
                if len(kept) > 1:
                    # Split: carrier Drains each take one wait.
                    for k, w in enumerate(kept[:-1]):
                        new_insts.append({
                            "name": f"{ins['name']}-w{k}",
                            "engine": eng, "ins": [], "outs": [],
                            "opcode": "Drain",
                            "sync_info": {"on_wait": [w], "on_update": []},
                        })
                        walk_idx += 1
                        comp_know.append(dict(prev_start_know.get(eng, {})))
                    kept = kept[-1:]
                if len(kept) != len(waits):
                    if not sy:
                        ins["sync_info"] = sy = {"on_update": []}
                    sy["on_wait"] = kept
                    waits = kept

            start_know = know_from(waits)
            prev_start_know[eng] = start_know

            own = set()
            for u in sy.get("on_update") or []:
                if (u.get("sync_type") == "semaphore"
                        and u.get("update_mode") == "sem-inc"):
                    s = u["id"]
                    cum[s] = cum.get(s, 0) + u.get("update_value", 1)
                    sem_reach.setdefault(s, []).append((cum[s], walk_idx))
                    own.add(s)
            ck = dict(start_know)
            for s in own:
                if usable(s) and len(contrib_engines.get(s, ())) == 1:
                    if ck.get(s, -1) < cum[s]:
                        ck[s] = cum[s]
            if eng in _COMPUTE_ENGINES:
                for s, c in cum.items():
                    if (usable(s) and contrib_engines.get(s) == {eng}
                            and ck.get(s, -1) < c):
                        ck[s] = c
            comp_know.append(ck)
            new_insts.append(ins)
            walk_idx += 1
        b["instructions"] = new_insts
    return bir


def _build_nc() -> bass.Bass:
    nc = bass.Bass()
    # Single fused input tensor: gt (replicated per 32-partition band)
    # in cols [0, M), pair tiles in cols [M, ABCOLS).
    ab = nc.declare_dram_parameter("ab", [128, ABCOLS], mybir.dt.bfloat16,
                                   isOutput=False)
    n_a = len(A_SET)
    n_b = NTILES - n_a
    res = nc.declare_dram_parameter("res", [128, NTILES], mybir.dt.float32,
                                    isOutput=True)

    f32 = mybir.dt.float32
    bf16 = mybir.dt.bfloat16
    f16 = mybir.dt.float16

    with tile.TileContext(nc) as tc:
        with (
            tc.tile_pool(name="const", bufs=1) as const_pool,
            tc.tile_pool(name="psum", bufs=2, space="PSUM") as psum_pool,
            tc.tile_pool(name="cp", bufs=3) as cp_pool,
        ):
            AB = const_pool.tile([128, ABCOLS], bf16, name="AB")
            AMINS = const_pool.tile([128, n_a], f32, name="AMINS")
            BMINS = const_pool.tile([128, n_b], f32, name="BMINS")
            # f32 keeps everything after it 4B-aligned (fp16 DVE 2x mode
            # requires 4B-aligned operands).
            WARM = const_pool.tile([128, 1], f32, name="WARM")

            # Pull the one-time ACT table load (~1.3us) off the critical
            # path: a dummy 1-col copy on memset data fires at t=0,
            # overlapping the input DMA.
            nc.gpsimd.memset(WARM[:, :], 0.0)
            nc.scalar.copy(WARM[:, :], WARM[:, :])

            # Input DMA ordered so tile 0's operands (gt chunks for its
            # two bands + first pair-block) land first.
            nc.sync.dma_start(out=AB[:, 0:512], in_=ab[:, 0:512])
            nc.sync.dma_start(out=AB[:, 1024:1536], in_=ab[:, 1024:1536])
            nc.sync.dma_start(out=AB[:, M:M + 256], in_=ab[:, M:M + 256])
            nc.sync.dma_start(out=AB[:, 512:1024], in_=ab[:, 512:1024])
            nc.sync.dma_start(out=AB[:, 1536:2048], in_=ab[:, 1536:2048])
            nc.sync.dma_start(out=AB[:, M + 256:M + 768],
                              in_=ab[:, M + 256:M + 768])
            nc.sync.dma_start(out=AB[:, M + 768:ABCOLS],
                              in_=ab[:, M + 768:ABCOLS])

            a_idx = 0
            b_idx = 0
            for t in range(NTILES):
                col = M + (t // 2) * 128
                ptile = psum_pool.tile([128, M], f32, tag="ptile")
                for hh in range(2):
                    q = (2 * t + hh) % 4
                    stat = AB[32 * q:32 * q + 32, col:col + 128]
                    for c in range(2):
                        gcol = 1024 * hh + 512 * c
                        nc.tensor.matmul(
                            out=ptile[:, gcol:gcol + 512],
                            lhsT=stat, rhs=AB[32 * q:32 * q + 32,
                                              gcol:gcol + 512],
                            start=True, stop=True,
                            tile_position=(32 * q, 0),
                        )
                if t in A_SET:
                    nc.vector.tensor_reduce(
                        out=AMINS[:, a_idx:a_idx + 1], in_=ptile[:, :],
                        axis=mybir.AxisListType.X, op=mybir.AluOpType.min,
                    )
                    a_idx += 1
                else:
                    stg = cp_pool.tile([128, M], f16, tag="stg")
                    nc.scalar.copy(stg[:, :], ptile[:, :])
                    nc.vector.tensor_reduce(
                        out=BMINS[:, b_idx:b_idx + 1], in_=stg[:, :],
                        axis=mybir.AxisListType.X, op=mybir.AluOpType.min,
                    )
                    b_idx += 1

            nc.sync.dma_start(out=res[:, 0:n_b], in_=BMINS[:, :])
            nc.sync.dma_start(out=res[:, n_b:NTILES], in_=AMINS[:, :])

    import json as _json

    pruned = _prune_redundant_waits(_json.loads(nc.to_json_bytes()))
    blob = _json.dumps(pruned).encode()
    nc.to_json_bytes = lambda: blob  # instance override read by bass2jax
    return nc


def _host_prep(recon_points: np.ndarray, gt_points: np.ndarray):
    """Build the per-core [128, ABCOLS] bf16 operand."""
    in_maps = []
    for core in range(N_CORES):
        b, h = divmod(core, 2)
        ii = _II[h * HALF:(h + 1) * HALF]
        jj = _JJ[h * HALF:(h + 1) * HALF]
        rec = recon_points[b].astype(np.float32)          # [64, 3]
        start, end = rec[ii], rec[jj]                     # [1008, 3]

        A = np.zeros((5, ROWS), dtype=np.float32)
        for fi, f in enumerate(FRACS):
            k = (start * np.float32(f) + end * np.float32(1.0 - f)).astype(np.float32)
            cols = slice(fi * HALF, (fi + 1) * HALF)
            A[0:3, cols] = k.T
            A[3, cols] = (k.astype(np.float64) ** 2).sum(1).astype(np.float32)
            A[4, cols] = 1.0
        ep = slice(NINT, NINT + NEND)
        re = rec[32 * h:32 * h + 32]
        A[0:3, ep] = re.T
        A[3, ep] = (re.astype(np.float64) ** 2).sum(1).astype(np.float32)
        A[4, ep] = 1.0

        g = gt_points[b].astype(np.float32)               # [2048, 3]
        Bm = np.empty((5, M), dtype=np.float32)
        Bm[0:3] = np.float32(-2.0) * g.T
        Bm[3] = 1.0
        Bm[4] = (g.astype(np.float64) ** 2).sum(1).astype(np.float32)

        Ah, Al, Ar = _split3_bf16(A)
        Bh, Bl, Br = _split3_bf16(Bm)
        # Product groups, largest magnitude first: hh | hl lh | hr rh ll
        A_ext = np.concatenate([Ah, Ah, Al, Ah, Ar, Al], axis=0)  # [30, ROWS]
        B_ext = np.concatenate([Bh, Bl, Bh, Br, Bh, Bl], axis=0)  # [30, M]

        import ml_dtypes
        bf16 = ml_dtypes.bfloat16
        ab = np.zeros((128, ABCOLS), dtype=bf16)
        for q in range(4):
            ab[32 * q:32 * q + 30, 0:M] = B_ext
        # Pair tiles: even t on partition bands 0 and 1, odd t on 2 and 3,
        # duplicated at both offsets its two halves use.
        for t in range(NTILES):
            colb = M + (t // 2) * 128
            base = 64 * (t % 2)
            blk = A_ext[:, 128 * t:128 * (t + 1)]
            ab[base:base + 30, colb:colb + 128] = blk
            ab[base + 32:base + 62, colb:colb + 128] = blk
        in_maps.append({"ab": np.ascontiguousarray(ab)})
    return in_maps


def _host_assemble(results) -> np.ndarray:
    n_a = len(A_SET)
    n_b = NTILES - n_a
    out = np.empty((B, P), dtype=np.float32)
    E_all = np.empty((B, NPTS), dtype=np.float32)
    s3 = {}
    for core in range(N_CORES):
        b, h = divmod(core, 2)
        res = np.asarray(results[core]["res"], dtype=np.float32)  # [128, 24]
        # res columns: [0:n_b] = B-tile mins (b-ordinal), [n_b:] = A-tile
        # mins (a-ordinal).
        tmins = np.empty((128, NTILES), dtype=np.float32)
        a_idx = b_idx = 0
        for t in range(NTILES):
            if t in A_SET:
                tmins[:, t] = res[:, n_b + a_idx]
                a_idx += 1
            else:
                tmins[:, t] = res[:, b_idx]
                b_idx += 1
        mins = tmins.T.reshape(-1)                # row r = 128*t + p
        s3[(b, h)] = mins[0:NINT].reshape(NF, HALF).sum(axis=0)
        E_all[b, 32 * h:32 * h + 32] = mins[NINT:NINT + NEND]
    for b in range(B):
        E = E_all[b]
        for h in range(2):
            sl = slice(h * HALF, (h + 1) * HALF)
            out[b, sl] = (s3[(b, h)] + E[_II[sl]] + E[_JJ[sl]]) * np.float32(0.2)
    return out


_NC_CACHE = None


def _get_nc() -> bass.Bass:
    global _NC_CACHE
    if _NC_CACHE is None:
        _NC_CACHE = _build_nc()
    return _NC_CACHE


def run(recon_points: np.ndarray, gt_points: np.ndarray, **spmd_kwargs):
    """Run on 8 NeuronCores; returns (output [4, 2016], BassKernelResults)."""
    nc = _get_nc()
    in_maps = _host_prep(recon_points, gt_points)
    r = run_bass_kernel_spmd(nc, in_maps, list(range(N_CORES)), **spmd_kwargs)
    return _host_assemble(r.results), r


def kernel(recon_points: np.ndarray, gt_points: np.ndarray) -> np.ndarray:
    recon_points = np.asarray(recon_points, dtype=np.float32)
    gt_points = np.asarray(gt_points, dtype=np.float32)
    out, _ = run(recon_points, gt_points)
    return out


# revision 9
# speedup vs baseline: 1.1740x; 1.1740x over previous
"""Trainium2 Bass kernel for nn_ComputeEdgeLoss.

Computes, for each batch b and lower-triangular pair (i, j) of the 64
recon keypoints, the mean over 5 interpolated segment points of the min
squared distance to the 2048 gt points of that batch.

Strategy (v3)
-------------
Sharding: 8 cores = 4 batches x 2 pair-halves (1008 pairs each); the 64
endpoint queries of a batch are split 32/32 between its two cores, so
each core owns exactly 3056 query rows -> 24 row-tiles of 128.  gt
replicated per batch (sharding_hint).

Math: for a query point k and gt point g,
    ||k - g||^2 = a . b,  a = [kx, ky, kz, ||k||^2, 1],
                          b = [-2gx, -2gy, -2gz, 1, ||g||^2]
so PE matmuls produce [128 x 2048] tiles of squared distances in PSUM.
Precision: fp32 inputs are split into three bf16 terms x ~= h + l + r
and the six product groups >= 2^-24 (hh, hl, lh, hr, rh, ll) become
K=30 bf16 contraction rows (padded to 32).  ~3e-4 max rel err measured.

PE array packing: K=32 <= 32 enables 4x row tiling -- four 32x128 bands
(tile_position=(32q, 0)) hold different stationary pair-blocks, each
streaming its own gt chunk.  Operands for band q live on SBUF
partitions [32q, 32q+32).

Drain (the bottleneck; walrus forbids dual-PSUM-operand TT, raw-ISA DVE
ops, Pool-engine tensor ops, and DMA min-accum, so only ScalarE+DVE can
touch the 6.3M PSUM values per core; measured rates: ScalarE copy
1 el/cyc @1.2GHz, DVE 1x for any PSUM/fp32/reduce op, 2x only for fp16
SBUF tensor_tensor -- tensor_reduce is 1x even on fp16):
 - A-tiles (1 in 4): DVE fp32 tensor_reduce(min) straight from PSUM,
   [128,2048] in one flat instruction (~2290ns measured).
 - B-tiles (3 in 4): ScalarE casts [128,2048] to fp16 in SBUF with a
   SINGLE copy (~1967ns; the v2 kernel paid 2x1113 for half copies),
   then a KB=3-batched fp16 TT fold pyramid at 2x (3D k-batched APs DO
   hit 2x: ~594ns per half-fold level) + one batched 3D reduce
   (~1234ns DVE per tile total).
Groups of (A,B,B,B) self-balance ScalarE (3x1967=5.9us) against DVE
(2290+3x1234=6.0us).  Scheduling (the v2 kernel idled DVE for the
first 16.6us): tile 0 is an A-tile so DVE works immediately; each
group's A-reduce is emitted BEFORE the previous group's folds so PSUM
buffers release promptly (DVE queue bias); PE (~21us at full p-state)
hides underneath the drain with a 2-deep [128,2048] PSUM ring.  A
dummy 1-col ScalarE copy at t=0 pulls the 1.3us ACT table load off the
critical path, the input DMA is ordered so tile 0's operands land
first, and all 24 per-tile mins leave in ONE output DMA (res col t =
tile t's min).
"""

import numpy as np

import concourse.bass as bass
import concourse.mybir as mybir
import concourse.tile as tile
from concourse.bass_utils import run_bass_kernel_spmd

# Problem shape (hardcoded per contest rules).
B = 4          # batches
NPTS = 64      # recon points per batch
M = 2048       # gt points per batch
P = NPTS * (NPTS - 1) // 2   # 2016 pairs
HALF = P // 2                # 1008 pairs per core
N_CORES = 8
FRACS = (0.25, 0.5, 0.75)    # interior interpolation fractions
NF = len(FRACS)
NINT = NF * HALF             # 3024 interior rows per core
NEND = 32                    # endpoint rows per core (64 split 32/32)
NTILES = 24                  # row-tiles of 128 (3072 rows, 3056 used)
ROWS = NTILES * 128
KEXT = 32                    # padded contraction depth (30 real rows)
PFCOLS = (NTILES // 2) * 128  # 1536: even tiles on partitions 0-63, odd on 64-127
ABCOLS = M + PFCOLS

_II, _JJ = np.tril_indices(NPTS, -1)   # pair order matches reference

# Tiles drained by DVE directly from PSUM (A); the rest are ScalarE-
# staged to fp16 then DVE-fold-reduced (B).  Pattern A,B,B,B per group
# of 4: DVE gets PSUM work the moment tile 0 lands while ScalarE copies
# the following tiles; each group is engine-balanced.
NGROUPS = NTILES // 4
A_SET = tuple(4 * g for g in range(NGROUPS))
KB = 3                       # B-tiles per fold group


def _split3_bf16(x: np.ndarray):
    """Split fp32 x into three bf16 terms with x ~= h + l + r (27-bit
    significand fidelity; differences are Sterbenz-exact in fp32)."""
    import ml_dtypes

    bf16 = ml_dtypes.bfloat16
    x = np.ascontiguousarray(x, dtype=np.float32)
    h = x.astype(bf16)
    l32 = (x - h.astype(np.float32)).astype(np.float32)
    l = l32.astype(bf16)
    r = (l32 - l.astype(np.float32)).astype(np.float32).astype(bf16)
    return h, l, r


_COMPUTE_ENGINES = {"PE", "DVE", "Activation", "Pool"}


def _prune_redundant_waits(bir: dict) -> dict:
    """Reduce every instruction to at most ONE sync-wait.

    This walrus build accepts only one sync-wait per instruction, but
    Tile's semaphore pass is not transitively minimal.  We reconstruct
    per-instruction guaranteed semaphore lower bounds (vector clocks
    over the scheduled program order) and delete implied waits; any
    residual multi-wait instruction is split into single-wait Drain
    carriers on the same engine.

    Soundness model: per-engine in-order dispatch; in-order completion
    for compute engines; per-semaphore in-order completion for DMA-queue
    sems (each DMAHW sem belongs to one queue).  Only monotone
    (inc-only) semaphores with sem-ge-imm waits participate.
    """
    fn = bir["functions"][0]

    # Semaphore properties across the whole program.
    contrib_engines: dict[int, set] = {}
    monotone: dict[int, bool] = {}
    for b in fn["blocks"]:
        for ins in b["instructions"]:
            sy = ins.get("sync_info") or {}
            for u in sy.get("on_update") or []:
                if u.get("sync_type") != "semaphore":
                    continue
                s = u["id"]
                contrib_engines.setdefault(s, set()).add(ins.get("engine"))
                ok = u.get("update_mode") == "sem-inc"
                monotone[s] = monotone.get(s, True) and ok

    def usable(s):
        return monotone.get(s, False)

    def mergemax(dst, src):
        for k, v in src.items():
            if dst.get(k, -1) < v:
                dst[k] = v

    prev_start_know: dict[str, dict] = {}
    cum: dict[int, int] = {}            # sem -> cumulative inc in walk order
    comp_know: list[dict] = []          # per walk index
    sem_reach: dict[int, list] = {}     # sem -> [(value_after, walk_idx)]
    dropped = 0
    walk_idx = 0

    for b in fn["blocks"]:
        new_insts = []
        for ins in b["instructions"]:
            eng = ins.get("engine")
            sy = ins.get("sync_info") or {}
            waits = list(sy.get("on_wait") or [])

            def know_from(wlist):
                know = dict(prev_start_know.get(eng, {}))
                for w in wlist:
                    if (w.get("sync_type") != "semaphore"
                            or w.get("wait_mode") != "sem-ge-imm"):
                        continue
                    s, v = w["id"], w["wait_value"]
                    if not usable(s):
                        continue
                    if know.get(s, -1) < v:
                        know[s] = v
                    if len(contrib_engines.get(s, ())) == 1:
                        for after, pidx in sem_reach.get(s, ()):
                            if after >= v:
                                mergemax(know, comp_know[pidx])
                                break
                return know

            if len(waits) > 1:
                kept = list(waits)
                changed = True
                while changed and len(kept) > 1:
                    changed = False
                    for w in list(kept):
                        others = [x for x in kept if x is not w]
                        if (w.get("sync_type") == "semaphore"
                                and w.get("wait_mode") == "sem-ge-imm"
                                and usable(w["id"])
                                and know_from(others).get(w["id"], -1)
                                >= w["wait_value"]):
                            kept.remove(w)
                            dropped += 1
                            changed = True
                            break
                if len(kept) > 1:
                    # Split: carrier Drains each take one wait.
                    for k, w in enumerate(kept[:-1]):
                        new_insts.append({
                            "name": f"{ins['name']}-w{k}",
                            "engine": eng, "ins": [], "outs": [],
                            "opcode": "Drain",
                            "sync_info": {"on_wait": [w], "on_update": []},
                        })
                        walk_idx += 1
                        comp_know.append(dict(prev_start_know.get(eng, {})))
                    kept = kept[-1:]
                if len(kept) != len(waits):
                    if not sy:
                        ins["sync_info"] = sy = {"on_update": []}
                    sy["on_wait"] = kept
                    waits = kept

            start_know = know_from(waits)
            prev_start_know[eng] = start_know

            own = set()
            for u in sy.get("on_update") or []:
                if (u.get("sync_type") == "semaphore"
                        and u.get("update_mode") == "sem-inc"):
                    s = u["id"]
                    cum[s] = cum.get(s, 0) + u.get("update_value", 1)
                    sem_reach.setdefault(s, []).append((cum[s], walk_idx))
                    own.add(s)
            ck = dict(start_know)
            for s in own:
                if usable(s) and len(contrib_engines.get(s, ())) == 1:
                    if ck.get(s, -1) < cum[s]:
                        ck[s] = cum[s]
            if eng in _COMPUTE_ENGINES:
                for s, c in cum.items():
                    if (usable(s) and contrib_engines.get(s) == {eng}
                            and ck.get(s, -1) < c):
                        ck[s] = c
            comp_know.append(ck)
            new_insts.append(ins)
            walk_idx += 1
        b["instructions"] = new_insts
    return bir


def _build_nc() -> bass.Bass:
    nc = bass.Bass()
    # Single fused input tensor: gt (replicated per 32-partition band)
    # in cols [0, M), pair tiles in cols [M, ABCOLS).
    ab = nc.declare_dram_parameter("ab", [128, ABCOLS], mybir.dt.bfloat16,
                                   isOutput=False)
    res = nc.declare_dram_parameter("res", [128, NTILES], mybir.dt.float32,
                                    isOutput=True)

    f32 = mybir.dt.float32
    bf16 = mybir.dt.bfloat16
    f16 = mybir.dt.float16

    with tile.TileContext(nc) as tc:
        with (
            tc.tile_pool(name="const", bufs=1) as const_pool,
            tc.tile_pool(name="psum", bufs=2, space="PSUM") as psum_pool,
            tc.tile_pool(name="cp", bufs=2) as cp_pool,
            tc.tile_pool(name="fold", bufs=2) as fold_pool,
        ):
            AB = const_pool.tile([128, ABCOLS], bf16, name="AB")
            # MINS col t = tile t's min; A-tiles at 4g, B-triples at
            # 4g+1..4g+3 so each group's batched fold-reduce writes 3
            # contiguous cols and ONE dma ships everything.
            MINS = const_pool.tile([128, NTILES], f32, name="MINS")
            WARM = const_pool.tile([128, 1], f32, name="WARM")

            # Pull the one-time ACT table load (~1.3us) off the critical
            # path: a dummy 1-col copy on memset data fires at t=0,
            # overlapping the input DMA.
            nc.gpsimd.memset(WARM[:, :], 0.0)
            nc.scalar.copy(WARM[:, :], WARM[:, :])

            # Input DMA ordered so tile 0's operands (gt chunks for its
            # two bands + first pair-blocks) land first.
            nc.sync.dma_start(out=AB[:, 0:512], in_=ab[:, 0:512])
            nc.sync.dma_start(out=AB[:, 1024:1536], in_=ab[:, 1024:1536])
            nc.sync.dma_start(out=AB[:, M:M + 256], in_=ab[:, M:M + 256])
            nc.sync.dma_start(out=AB[:, 512:1024], in_=ab[:, 512:1024])
            nc.sync.dma_start(out=AB[:, 1536:2048], in_=ab[:, 1536:2048])
            nc.sync.dma_start(out=AB[:, M + 256:M + 768],
                              in_=ab[:, M + 256:M + 768])
            nc.sync.dma_start(out=AB[:, M + 768:ABCOLS],
                              in_=ab[:, M + 768:ABCOLS])

            def emit_tile(t):
                col = M + (t // 2) * 128
                ptile = psum_pool.tile([128, M], f32, tag="ptile")
                for hh in range(2):
                    q = (2 * t + hh) % 4
                    stat = AB[32 * q:32 * q + 32, col:col + 128]
                    for c in range(2):
                        gcol = 1024 * hh + 512 * c
                        nc.tensor.matmul(
                            out=ptile[:, gcol:gcol + 512],
                            lhsT=stat, rhs=AB[32 * q:32 * q + 32,
                                              gcol:gcol + 512],
                            start=True, stop=True,
                            tile_position=(32 * q, 0),
                        )
                return ptile

            def emit_folds(cp_tile, b0):
                """KB-batched fp16 TT fold pyramid + batched reduce for
                B-tiles at MINS cols [b0, b0+KB)."""
                c3 = cp_tile[:, :].rearrange("p (k n) -> p k n", n=M)
                j1 = fold_pool.tile([128, KB * (M // 2)], f16, tag="j1")
                v1 = j1[:, :].rearrange("p (k n) -> p k n", n=M // 2)
                nc.vector.tensor_tensor(
                    out=v1, in0=c3[:, :, 0:M // 2], in1=c3[:, :, M // 2:M],
                    op=mybir.AluOpType.min)
                j2 = fold_pool.tile([128, KB * (M // 4)], f16, tag="j2")
                v2 = j2[:, :].rearrange("p (k n) -> p k n", n=M // 4)
                nc.vector.tensor_tensor(
                    out=v2, in0=v1[:, :, 0:M // 4], in1=v1[:, :, M // 4:M // 2],
                    op=mybir.AluOpType.min)
                j3 = fold_pool.tile([128, KB * (M // 8)], f16, tag="j3")
                v3 = j3[:, :].rearrange("p (k n) -> p k n", n=M // 8)
                nc.vector.tensor_tensor(
                    out=v3, in0=v2[:, :, 0:M // 8], in1=v2[:, :, M // 8:M // 4],
                    op=mybir.AluOpType.min)
                j4 = fold_pool.tile([128, KB * (M // 16)], f16, tag="j4")
                v4 = j4[:, :].rearrange("p (k n) -> p k n", n=M // 16)
                nc.vector.tensor_tensor(
                    out=v4, in0=v3[:, :, 0:M // 16], in1=v3[:, :, M // 16:M // 8],
                    op=mybir.AluOpType.min)
                nc.vector.tensor_reduce(
                    out=MINS[:, b0:b0 + KB], in_=v4,
                    axis=mybir.AxisListType.X, op=mybir.AluOpType.min,
                )

            # Software-pipelined drain: per group g emit the A-reduce
            # FIRST (so it jumps ahead of fold work in the DVE queue and
            # releases its PSUM buffer promptly), then the three Scalar
            # copies, then the PREVIOUS group's folds.
            pending = None   # (cp_tile, b0) of the previous group
            for g in range(NGROUPS):
                pt = emit_tile(4 * g)
                nc.vector.tensor_reduce(
                    out=MINS[:, 4 * g:4 * g + 1], in_=pt[:, :],
                    axis=mybir.AxisListType.X, op=mybir.AluOpType.min,
                )
                cp_cur = cp_pool.tile([128, KB * M], f16, tag="cp")
                for j in range(KB):
                    pt = emit_tile(4 * g + 1 + j)
                    nc.scalar.copy(cp_cur[:, j * M:(j + 1) * M], pt[:, :])
                if pending is not None:
                    emit_folds(*pending)
                pending = (cp_cur, 4 * g + 1)
            emit_folds(*pending)

            nc.sync.dma_start(out=res[:, :], in_=MINS[:, :])

    import json as _json

    pruned = _prune_redundant_waits(_json.loads(nc.to_json_bytes()))
    blob = _json.dumps(pruned).encode()
    nc.to_json_bytes = lambda: blob  # instance override read by bass2jax
    return nc


def _host_prep(recon_points: np.ndarray, gt_points: np.ndarray):
    """Build the per-core [128, ABCOLS] bf16 operand."""
    in_maps = []
    for core in range(N_CORES):
        b, h = divmod(core, 2)
        ii = _II[h * HALF:(h + 1) * HALF]
        jj = _JJ[h * HALF:(h + 1) * HALF]
        rec = recon_points[b].astype(np.float32)          # [64, 3]
        start, end = rec[ii], rec[jj]                     # [1008, 3]

        A = np.zeros((5, ROWS), dtype=np.float32)
        for fi, f in enumerate(FRACS):
            k = (start * np.float32(f) + end * np.float32(1.0 - f)).astype(np.float32)
            cols = slice(fi * HALF, (fi + 1) * HALF)
            A[0:3, cols] = k.T
            A[3, cols] = (k.astype(np.float64) ** 2).sum(1).astype(np.float32)
            A[4, cols] = 1.0
        ep = slice(NINT, NINT + NEND)
        re = rec[32 * h:32 * h + 32]
        A[0:3, ep] = re.T
        A[3, ep] = (re.astype(np.float64) ** 2).sum(1).astype(np.float32)
        A[4, ep] = 1.0

        g = gt_points[b].astype(np.float32)               # [2048, 3]
        Bm = np.empty((5, M), dtype=np.float32)
        Bm[0:3] = np.float32(-2.0) * g.T
        Bm[3] = 1.0
        Bm[4] = (g.astype(np.float64) ** 2).sum(1).astype(np.float32)

        Ah, Al, Ar = _split3_bf16(A)
        Bh, Bl, Br = _split3_bf16(Bm)
        # Product groups, largest magnitude first: hh | hl lh | hr rh ll
        A_ext = np.concatenate([Ah, Ah, Al, Ah, Ar, Al], axis=0)  # [30, ROWS]
        B_ext = np.concatenate([Bh, Bl, Bh, Br, Bh, Bl], axis=0)  # [30, M]

        import ml_dtypes
        bf16 = ml_dtypes.bfloat16
        ab = np.zeros((128, ABCOLS), dtype=bf16)
        for q in range(4):
            ab[32 * q:32 * q + 30, 0:M] = B_ext
        # Pair tiles: even t on partition bands 0 and 1, odd t on 2 and 3,
        # duplicated at both offsets its two halves use.
        for t in range(NTILES):
            colb = M + (t // 2) * 128
            base = 64 * (t % 2)
            blk = A_ext[:, 128 * t:128 * (t + 1)]
            ab[base:base + 30, colb:colb + 128] = blk
            ab[base + 32:base + 62, colb:colb + 128] = blk
        in_maps.append({"ab": np.ascontiguousarray(ab)})
    return in_maps


def _host_assemble(results) -> np.ndarray:
    out = np.empty((B, P), dtype=np.float32)
    E_all = np.empty((B, NPTS), dtype=np.float32)
    s3 = {}
    for core in range(N_CORES):
        b, h = divmod(core, 2)
        res = np.asarray(results[core]["res"], dtype=np.float32)  # [128, 24]
        mins = res.T.reshape(-1)                  # row r = 128*t + p
        s3[(b, h)] = mins[0:NINT].reshape(NF, HALF).sum(axis=0)
        E_all[b, 32 * h:32 * h + 32] = mins[NINT:NINT + NEND]
    for b in range(B):
        E = E_all[b]
        for h in range(2):
            sl = slice(h * HALF, (h + 1) * HALF)
            out[b, sl] = (s3[(b, h)] + E[_II[sl]] + E[_JJ[sl]]) * np.float32(0.2)
    return out


_NC_CACHE = None


def _get_nc() -> bass.Bass:
    global _NC_CACHE
    if _NC_CACHE is None:
        _NC_CACHE = _build_nc()
    return _NC_CACHE


def run(recon_points: np.ndarray, gt_points: np.ndarray, **spmd_kwargs):
    """Run on 8 NeuronCores; returns (output [4, 2016], BassKernelResults)."""
    nc = _get_nc()
    in_maps = _host_prep(recon_points, gt_points)
    r = run_bass_kernel_spmd(nc, in_maps, list(range(N_CORES)), **spmd_kwargs)
    return _host_assemble(r.results), r


def kernel(recon_points: np.ndarray, gt_points: np.ndarray) -> np.ndarray:
    recon_points = np.asarray(recon_points, dtype=np.float32)
    gt_points = np.asarray(gt_points, dtype=np.float32)
    out, _ = run(recon_points, gt_points)
    return out
